# revision 6
# baseline (speedup 1.0000x reference)
"""MHA kernel for TRN2: B=4,T=2048,D=1024,H=16,HD=64 across 8 NeuronCores.

Sharding: core c -> batch c//2, head-half c%2 (8 heads per core, all 2048
queries). Host pre-transposes and pre-casts all operands to bf16 so the
device does zero transposes; each core returns a partial output (its 8 heads
through the matching 512 rows of Wo^T) and the host sums the pair + bias.

Device layout: everything keeps d (or cat-dim) on partitions. Logits are
computed transposed (P^T [s,q]) so softmax denominators come from a
ones-column folded into V; denominators broadcast across partitions with a
K=1 matmul. All matmuls bf16 (fp32 PSUM accumulate).
"""
import sys
sys.path.insert(0, "/opt/trn_rl_repo")
import warnings
warnings.filterwarnings("ignore")

import numpy as np
import ml_dtypes
import concourse.bass as bass
import concourse.mybir as mybir
import concourse.tile as tile
from concourse import bacc
from concourse.bass_utils import run_bass_kernel_spmd

F32 = mybir.dt.float32
F32R = mybir.dt.float32r
BF16 = mybir.dt.bfloat16
EXP = mybir.ActivationFunctionType.Exp

B, T, D, H = 4, 2048, 1024, 16
NH = 8             # heads per core
NG = 4             # head groups of 2
NSC = 16           # s chunks of 128
NDC = 8            # d chunks of 128
SCALE = 0.125      # 1/sqrt(64)
BF = np.dtype(ml_dtypes.bfloat16)


def build_nc():
    nc = bacc.Bacc("TRN2", target_bir_lowering=False, debug=False, num_devices=8)
    xt = nc.dram_tensor("xt", [D, T], BF16, kind="ExternalInput")
    wqt = nc.dram_tensor("wqt", [D, 512], BF16, kind="ExternalInput")
    wkt = nc.dram_tensor("wkt", [D, 512], BF16, kind="ExternalInput")
    wvt = nc.dram_tensor("wvt", [D, 512], BF16, kind="ExternalInput")
    wot = nc.dram_tensor("wot", [512, D], BF16, kind="ExternalInput")
    y = nc.dram_tensor("y", [T, D], F32, kind="ExternalOutput")

    with tile.TileContext(nc) as tc:
        with (
            tc.tile_pool(name="persist", bufs=1) as pp,
            tc.tile_pool(name="qk", bufs=2) as qkp,
            tc.tile_pool(name="ptp", bufs=3) as ptp,
            tc.tile_pool(name="small", bufs=2) as sp,
            tc.tile_pool(name="yt", bufs=2) as ytp,
            tc.tile_pool(name="ps_work", bufs=2, space="PSUM") as psw,
            tc.tile_pool(name="ps_pv", bufs=2, space="PSUM") as psv,
            tc.tile_pool(name="ps_log", bufs=2, space="PSUM") as psl,
        ):
            onesf = pp.tile([65, 64], F32)
            nc.vector.memset(onesf, 1.0)
            ones = pp.tile([65, 64], F32R)
            nc.vector.tensor_copy(out=ones, in_=onesf)

            xT = pp.tile([128, NDC, T], BF16, name="xT")
            wq = pp.tile([128, NDC, 512], BF16, name="wq")
            wk = pp.tile([128, NDC, 512], BF16, name="wk")
            wv = pp.tile([128, NDC, 512], BF16, name="wv")
            wo = pp.tile([128, NG, D], BF16, name="wo")
            vt = pp.tile([128, NSC, NH, 65], BF16, name="vt")
            catT = pp.tile([128, NG, T], BF16, name="catT")

            for dc in range(NDC):
                nc.sync.dma_start(out=xT[:, dc, :], in_=xt[dc * 128:(dc + 1) * 128, :])
            for (src, dst) in ((wqt, wq), (wkt, wk), (wvt, wv)):
                for dc in range(NDC):
                    nc.sync.dma_start(
                        out=dst[:, dc, :], in_=src[dc * 128:(dc + 1) * 128, :])
            for rb in range(NG):
                nc.sync.dma_start(out=wo[:, rb, :], in_=wot[rb * 128:(rb + 1) * 128, :])
            nc.vector.memset(vt[:, :, :, 64:65], 1.0)

            def v_chain(sc):
                p = psw.tile([128, 512], F32, tag="work")
                for dc in range(NDC):
                    nc.tensor.matmul(
                        p, xT[:, dc, sc * 128:(sc + 1) * 128], wv[:, dc, :],
                        start=(dc == 0), stop=(dc == NDC - 1))
                nc.vector.tensor_copy(
                    out=vt[:, sc, :, 0:64],
                    in_=p.rearrange("p (h c) -> p h c", h=NH))

            def qk_chain(g, qi, qc4):
                src = wq if qi == 0 else wk
                qkT = qk_tiles[g]
                p = psw.tile([128, 512], F32, tag="work")
                for dc in range(NDC):
                    nc.tensor.matmul(
                        p, src[:, dc, g * 128:(g + 1) * 128],
                        xT[:, dc, qc4 * 512:(qc4 + 1) * 512],
                        start=(dc == 0), stop=(dc == NDC - 1))
                nc.vector.tensor_copy(
                    out=qkT[:, qi, qc4 * 512:(qc4 + 1) * 512], in_=p)

            def o_chain(qb):
                yt = ytp.tile([128, D], F32, tag="yt")
                for nh in range(2):
                    p = psw.tile([128, 512], F32, tag="work")
                    for g in range(NG):
                        nc.tensor.matmul(
                            p, catT[:, g, qb * 128:(qb + 1) * 128],
                            wo[:, g, nh * 512:(nh + 1) * 512],
                            start=(g == 0), stop=(g == NG - 1))
                    nc.vector.tensor_copy(out=yt[:, nh * 512:(nh + 1) * 512], in_=p)
                nc.sync.dma_start(out=y[qb * 128:(qb + 1) * 128, :], in_=yt)

            qk_tiles = {}
            # Pre-build Q/K for group 0 so the first attention block can start.
            qk_tiles[0] = qkp.tile([128, 2, T], BF16, tag="qkT", name="qkT0")
            for qi in range(2):
                for qc4 in range(4):
                    qk_chain(0, qi, qc4)

            for g in range(NG):
                # next group's Q/K build is interleaved into this group's
                # attention iterations as filler work for the PE
                fillers = []
                if g + 1 < NG:
                    qk_tiles[g + 1] = qkp.tile(
                        [128, 2, T], BF16, tag="qkT", name=f"qkT{g+1}")
                    fillers = [
                        (lambda gg=g + 1, qi=qi, qc4=qc4: qk_chain(gg, qi, qc4))
                        for qi in range(2) for qc4 in range(4)]
                qkT = qk_tiles[g]
                n_fill_iter = 3 * NSC if g == 0 else 4 * NSC
                stride = max(1, n_fill_iter // max(1, len(fillers)))
                fi = 0
                it = 0
                for qc in range(4):
                    qs = slice(qc * 512, (qc + 1) * 512)
                    pv0 = psv.tile([65, 512], F32, tag="pv")
                    pv1 = psv.tile([65, 512], F32, tag="pv")
                    for sc in range(NSC):
                        if g == 0 and qc == 0:
                            # V for chunk sc must exist before its PV matmul.
                            v_chain(sc)
                        else:
                            if fi < len(fillers) and it % stride == 0:
                                fillers[fi]()
                                fi += 1
                            it += 1
                        lg = psl.tile([128, 2, 512], F32, tag="log")
                        nc.tensor.matmul(
                            lg[:, 0, :], qkT[0:64, 1, sc * 128:(sc + 1) * 128],
                            qkT[0:64, 0, qs], start=True, stop=True)
                        nc.tensor.matmul(
                            lg[:, 1, :], qkT[64:128, 1, sc * 128:(sc + 1) * 128],
                            qkT[64:128, 0, qs], start=True, stop=True)
                        pt = ptp.tile([128, 2, 512], BF16, tag="pt")
                        nc.scalar.activation(
                            out=pt.rearrange("p a b -> p (a b)"),
                            in_=lg.rearrange("p a b -> p (a b)"),
                            func=EXP, scale=SCALE)
                        nc.tensor.matmul(
                            pv0, vt[:, sc, 2 * g, :], pt[:, 0, :],
                            start=(sc == 0), stop=(sc == NSC - 1))
                        nc.tensor.matmul(
                            pv1, vt[:, sc, 2 * g + 1, :], pt[:, 1, :],
                            start=(sc == 0), stop=(sc == NSC - 1))
                    for hloc, pv in ((0, pv0), (1, pv1)):
                        # DVE lanes are per-partition: reads/writes must stay
                        # on the same partitions, so the denominator row is
                        # copied at partition 64 and head 1's normalized tile
                        # reaches partitions 64-127 via DMA.
                        s1 = sp.tile([65, 512], F32R, tag="s1")
                        nc.vector.tensor_copy(out=s1[64:65, :], in_=pv[64:65, :])
                        pb = psw.tile([128, 512], F32, tag="work")
                        nc.tensor.matmul(
                            pb[0:64, :], ones[64:65, :], s1[64:65, :],
                            start=True, stop=True)
                        rec = sp.tile([64, 512], F32, tag="rec")
                        nc.vector.reciprocal(out=rec, in_=pb[0:64, :])
                        if hloc == 0:
                            nc.vector.tensor_mul(
                                out=catT[0:64, g, qs], in0=pv[0:64, :], in1=rec)
                        else:
                            tmp = sp.tile([64, 512], BF16, tag="tmp")
                            nc.vector.tensor_mul(out=tmp, in0=pv[0:64, :], in1=rec)
                            nc.sync.dma_start(out=catT[64:128, g, qs], in_=tmp)
                    if g == NG - 1:
                        for qb in range(qc * 4, (qc + 1) * 4):
                            o_chain(qb)

    nc.compile()
    return nc


_CACHE = {}


def _prep(x, Wq, Wk, Wv, Wo):
    """Host-side: transpose + cast to bf16, split by head-half."""
    xts = [np.ascontiguousarray(x[b].T.astype(BF)) for b in range(B)]
    halves = []
    for hh in range(2):
        sl = slice(hh * (H // 2), (hh + 1) * (H // 2))
        wqt = np.ascontiguousarray(Wq[sl].reshape(512, D).T.astype(BF))
        wkt = np.ascontiguousarray(Wk[sl].reshape(512, D).T.astype(BF))
        wvt = np.ascontiguousarray(Wv[sl].reshape(512, D).T.astype(BF))
        wot = np.ascontiguousarray(Wo[:, hh * 512:(hh + 1) * 512].T.astype(BF))
        halves.append((wqt, wkt, wvt, wot))
    return xts, halves


def kernel(x, Wq, Wk, Wv, Wo, bo):
    if "nc" not in _CACHE:
        _CACHE["nc"] = build_nc()
    nc = _CACHE["nc"]
    x = np.asarray(x, dtype=np.float32)
    xts, halves = _prep(
        x, np.asarray(Wq, np.float32), np.asarray(Wk, np.float32),
        np.asarray(Wv, np.float32), np.asarray(Wo, np.float32))
    in_maps = []
    for c in range(8):
        b, hh = c // 2, c % 2
        wqt, wkt, wvt, wot = halves[hh]
        in_maps.append({"xt": xts[b], "wqt": wqt, "wkt": wkt, "wvt": wvt,
                        "wot": wot})
    res = run_bass_kernel_spmd(nc, in_maps, core_ids=list(range(8)))
    bo2 = np.asarray(bo, np.float32).reshape(1, D)
    out = np.empty((B, T, D), dtype=np.float32)
    for b in range(B):
        out[b] = res.results[2 * b]["y"] + res.results[2 * b + 1]["y"] + bo2
    return out


# revision 21
# speedup vs baseline: 1.0693x; 1.0693x over previous
"""MHA kernel for TRN2: B=4,T=2048,D=1024,H=16,HD=64 across 8 NeuronCores.

Sharding: core c -> batch c//2, head-half c%2 (8 heads per core, all 2048
queries). Host pre-transposes and pre-casts all operands to bf16 so the
device does zero transposes; each core returns a partial output (its 8 heads
through the matching 512 rows of Wo^T) and the host sums the pair + bias.

Device layout: everything keeps d (or cat-dim) on partitions. Logits are
computed transposed (P^T [s,q]) so softmax denominators come from a
ones-column folded into V; denominators broadcast across partitions with a
K=1 matmul. All matmuls bf16 (fp32 PSUM accumulate).
"""
import sys
sys.path.insert(0, "/opt/trn_rl_repo")
import warnings
warnings.filterwarnings("ignore")

import numpy as np
import ml_dtypes
import concourse.bass as bass
import concourse.mybir as mybir
import concourse.tile as tile
from concourse import bacc
from concourse.bass_utils import run_bass_kernel_spmd

F32 = mybir.dt.float32
F32R = mybir.dt.float32r
BF16 = mybir.dt.bfloat16
EXP = mybir.ActivationFunctionType.Exp

B, T, D, H = 4, 2048, 1024, 16
NH = 8             # heads per core
NG = 4             # head groups of 2
NSC = 16           # s chunks of 128
NDC = 8            # d chunks of 128
SCALE = 0.125      # 1/sqrt(64)
BF = np.dtype(ml_dtypes.bfloat16)


def build_nc():
    nc = bacc.Bacc("TRN2", target_bir_lowering=False, debug=False, num_devices=8)
    xt = nc.dram_tensor("xt", [D, T], BF16, kind="ExternalInput")
    wqt = nc.dram_tensor("wqt", [D, 512], BF16, kind="ExternalInput")
    wkt = nc.dram_tensor("wkt", [D, 512], BF16, kind="ExternalInput")
    wvt = nc.dram_tensor("wvt", [D, 512], BF16, kind="ExternalInput")
    wot = nc.dram_tensor("wot", [512, D], BF16, kind="ExternalInput")
    y = nc.dram_tensor("y", [T, D], BF16, kind="ExternalOutput")

    with tile.TileContext(nc) as tc:
        with (
            tc.tile_pool(name="persist", bufs=1) as pp,
            tc.tile_pool(name="qk", bufs=2) as qkp,
            tc.tile_pool(name="ptp", bufs=3) as ptp,
            tc.tile_pool(name="small", bufs=2) as sp,
            tc.tile_pool(name="yt", bufs=2) as ytp,
            tc.tile_pool(name="ps_work", bufs=2, space="PSUM") as psw,
            tc.tile_pool(name="ps_pv", bufs=2, space="PSUM") as psv,
            tc.tile_pool(name="ps_log", bufs=2, space="PSUM") as psl,
        ):
            onesf = pp.tile([65, 64], F32)
            nc.vector.memset(onesf, 1.0)
            ones = pp.tile([65, 64], F32R)
            nc.vector.tensor_copy(out=ones, in_=onesf)

            xT = pp.tile([128, NDC, T], BF16, name="xT")
            wq = pp.tile([128, NDC, 512], BF16, name="wq")
            wk = pp.tile([128, NDC, 512], BF16, name="wk")
            wv = pp.tile([128, NDC, 512], BF16, name="wv")
            wo = pp.tile([128, NG, D], BF16, name="wo")
            vt = pp.tile([128, NSC, NH, 65], BF16, name="vt")
            catT = pp.tile([128, NG, T], BF16, name="catT")

            # x streams in densely first: every projection chain contracts
            # over ALL of x, so the first chain finishes no earlier than the
            # last x chunk — don't dilute x bandwidth with weight loads.
            # Weights for Q/K (needed first) follow, then V, then Wo.
            for dc in range(NDC):
                nc.sync.dma_start(out=xT[:, dc, :], in_=xt[dc * 128:(dc + 1) * 128, :])
            for (src, dst) in ((wqt, wq), (wkt, wk), (wvt, wv)):
                for dc in range(NDC):
                    nc.sync.dma_start(
                        out=dst[:, dc, :], in_=src[dc * 128:(dc + 1) * 128, :])
            for rb in range(NG):
                nc.sync.dma_start(out=wo[:, rb, :], in_=wot[rb * 128:(rb + 1) * 128, :])
            nc.vector.memset(vt[:, :, :, 64:65], 1.0)

            def v_chain(sc):
                """Generator: one matmul per step (spreadable filler work)."""
                p = psw.tile([128, 512], F32, tag="work")
                for dc in range(NDC):
                    nc.tensor.matmul(
                        p, xT[:, dc, sc * 128:(sc + 1) * 128], wv[:, dc, :],
                        start=(dc == 0), stop=(dc == NDC - 1))
                    if dc < NDC - 1:
                        yield
                nc.vector.tensor_copy(
                    out=vt[:, sc, :, 0:64],
                    in_=p.rearrange("p (h c) -> p h c", h=NH))
                yield

            def qk_chain(g, qi, qc4):
                src = wq if qi == 0 else wk
                qkT = qk_tiles[g]
                p = psw.tile([128, 512], F32, tag="work")
                for dc in range(NDC):
                    nc.tensor.matmul(
                        p, src[:, dc, g * 128:(g + 1) * 128],
                        xT[:, dc, qc4 * 512:(qc4 + 1) * 512],
                        start=(dc == 0), stop=(dc == NDC - 1))
                    if dc < NDC - 1:
                        yield
                nc.vector.tensor_copy(
                    out=qkT[:, qi, qc4 * 512:(qc4 + 1) * 512], in_=p)
                yield

            def o_chain(qb):
                yt = ytp.tile([128, D], BF16, tag="yt")
                for nh in range(2):
                    p = psw.tile([128, 512], F32, tag="work")
                    for g in range(NG):
                        nc.tensor.matmul(
                            p, catT[:, g, qb * 128:(qb + 1) * 128],
                            wo[:, g, nh * 512:(nh + 1) * 512],
                            start=(g == 0), stop=(g == NG - 1))
                        if g < NG - 1:
                            yield
                    nc.vector.tensor_copy(out=yt[:, nh * 512:(nh + 1) * 512], in_=p)
                    yield
                nc.sync.dma_start(out=y[qb * 128:(qb + 1) * 128, :], in_=yt)
                yield

            qk_tiles = {}
            # Pre-build Q/K for group 0 so the first attention block can start.
            qk_tiles[0] = qkp.tile([128, 2, T], BF16, tag="qkT", name="qkT0")
            for qi in range(2):
                for qc4 in range(4):
                    for _ in qk_chain(0, qi, qc4):
                        pass

            # Filler scheduler: spreads long matmul chains one-matmul-per-step
            # through the ACT-bound attention iterations so the PE never
            # bursts then stalls. `credit` accumulates a fractional per-call
            # rate so the work spreads evenly instead of front-loading.
            fill_state = {"active": None, "pending": [], "credit": 0.0}

            def drain(n):
                fill_state["credit"] += n
                while fill_state["credit"] >= 1.0:
                    if fill_state["active"] is None:
                        if not fill_state["pending"]:
                            fill_state["credit"] = 0.0
                            return
                        fill_state["active"] = fill_state["pending"].pop(0)
                    try:
                        next(fill_state["active"])
                        fill_state["credit"] -= 1.0
                    except StopIteration:
                        fill_state["active"] = None

            def make_norm(g, qs, qc, pvs0, pvs1):
                """Normalize a finished block's PV results (already copied to
                SBUF) into catT. Deferred into the next block so the PE/ACT
                front of that block isn't serialized behind this DVE chain."""
                def norm():
                    for hloc, pvs in ((0, pvs0), (1, pvs1)):
                        # DVE lanes are per-partition: reads/writes must stay
                        # on the same partitions, so head 1's normalized tile
                        # reaches partitions 64-127 via DMA.
                        pb = psw.tile([128, 512], F32, tag="work")
                        nc.tensor.matmul(
                            pb[0:64, :], ones[64:65, :], pvs[64:65, :],
                            start=True, stop=True)
                        rec = sp.tile([64, 512], F32, tag="rec")
                        nc.vector.reciprocal(out=rec, in_=pb[0:64, :])
                        if hloc == 0:
                            nc.vector.tensor_mul(
                                out=catT[0:64, g, qs], in0=pvs[0:64, :], in1=rec)
                        else:
                            tmp = sp.tile([64, 512], BF16, tag="tmp")
                            nc.vector.tensor_mul(out=tmp, in0=pvs[0:64, :], in1=rec)
                            nc.sync.dma_start(out=catT[64:128, g, qs], in_=tmp)
                    if g == NG - 1:
                        # output projection for the finished rows becomes
                        # filler work
                        fill_state["pending"].extend(
                            o_chain(qb) for qb in range(qc * 4, (qc + 1) * 4))
                return norm

            pending_norm = None
            for g in range(NG):
                # next group's Q/K build is interleaved into this group's
                # attention iterations as filler work for the PE
                if g + 1 < NG:
                    qk_tiles[g + 1] = qkp.tile(
                        [128, 2, T], BF16, tag="qkT", name=f"qkT{g+1}")
                    fill_state["pending"].extend(
                        qk_chain(g + 1, qi, qc4)
                        for qi in range(2) for qc4 in range(4))
                qkT = qk_tiles[g]
                # spread filler steps evenly: qk chains (72 steps) over the
                # fillable iters of this group; o_chains (g=3) arrive at 11
                # steps per finished qc block drained over the next 16 iters
                nfill = 1.2 if g < NG - 1 else 2.8
                for qc in range(4):
                    qs = slice(qc * 512, (qc + 1) * 512)
                    if g == 0 and qc == 0:
                        for _ in v_chain(0):
                            pass
                    pv0 = psv.tile([65, 512], F32, tag="pv")
                    pv1 = psv.tile([65, 512], F32, tag="pv")

                    def pv_step(sc, pts, pv0=pv0, pv1=pv1, g=g):
                        nc.tensor.matmul(
                            pv0, vt[:, sc, 2 * g, :], pts[:, 0, :],
                            start=(sc == 0), stop=(sc == NSC - 1))
                        nc.tensor.matmul(
                            pv1, vt[:, sc, 2 * g + 1, :], pts[:, 1, :],
                            start=(sc == 0), stop=(sc == NSC - 1))

                    # software pipeline: PV for chunk sc-1 is emitted before
                    # the logits of chunk sc (its input is already available,
                    # while the logits may wait on an ACT buffer), so the PE
                    # always has runnable work at the queue head.
                    prev = None
                    for sc in range(NSC):
                        if prev is not None:
                            pv_step(*prev)
                        lg = psl.tile([128, 2, 512], F32, tag="log")
                        nc.tensor.matmul(
                            lg[:, 0, :], qkT[0:64, 1, sc * 128:(sc + 1) * 128],
                            qkT[0:64, 0, qs], start=True, stop=True)
                        nc.tensor.matmul(
                            lg[:, 1, :], qkT[64:128, 1, sc * 128:(sc + 1) * 128],
                            qkT[64:128, 0, qs], start=True, stop=True)
                        pt = ptp.tile([128, 2, 512], BF16, tag="pt")
                        nc.scalar.activation(
                            out=pt.rearrange("p a b -> p (a b)"),
                            in_=lg.rearrange("p a b -> p (a b)"),
                            func=EXP, scale=SCALE)
                        prev = (sc, pt)
                        if sc == 0 and pending_norm is not None:
                            # previous block's normalization, after this
                            # block's first logits are already in flight
                            pending_norm()
                            pending_norm = None
                        if g == 0 and qc == 0:
                            # V for chunk sc+1 must exist before its PV matmul.
                            if sc + 1 < NSC:
                                for _ in v_chain(sc + 1):
                                    pass
                        else:
                            drain(nfill)
                    pv_step(*prev)
                    # copy PV accumulators to SBUF right away: one DVE op per
                    # bank frees the PSUM for the next block's accumulation
                    pvs0 = sp.tile([65, 512], F32R, tag="pvs0")
                    pvs1 = sp.tile([65, 512], F32R, tag="pvs1")
                    nc.vector.tensor_copy(out=pvs0, in_=pv0)
                    nc.vector.tensor_copy(out=pvs1, in_=pv1)
                    pending_norm = make_norm(g, qs, qc, pvs0, pvs1)
                if g < NG - 1:
                    drain(10 ** 9)
            pending_norm()
            drain(10 ** 9)

    nc.compile()
    return nc


_CACHE = {}


def _prep(x, Wq, Wk, Wv, Wo):
    """Host-side: transpose + cast to bf16, split by head-half."""
    xts = [np.ascontiguousarray(x[b].T.astype(BF)) for b in range(B)]
    halves = []
    for hh in range(2):
        sl = slice(hh * (H // 2), (hh + 1) * (H // 2))
        wqt = np.ascontiguousarray(Wq[sl].reshape(512, D).T.astype(BF))
        wkt = np.ascontiguousarray(Wk[sl].reshape(512, D).T.astype(BF))
        wvt = np.ascontiguousarray(Wv[sl].reshape(512, D).T.astype(BF))
        wot = np.ascontiguousarray(Wo[:, hh * 512:(hh + 1) * 512].T.astype(BF))
        halves.append((wqt, wkt, wvt, wot))
    return xts, halves


def kernel(x, Wq, Wk, Wv, Wo, bo):
    if "nc" not in _CACHE:
        _CACHE["nc"] = build_nc()
    nc = _CACHE["nc"]
    x = np.asarray(x, dtype=np.float32)
    xts, halves = _prep(
        x, np.asarray(Wq, np.float32), np.asarray(Wk, np.float32),
        np.asarray(Wv, np.float32), np.asarray(Wo, np.float32))
    in_maps = []
    for c in range(8):
        b, hh = c // 2, c % 2
        wqt, wkt, wvt, wot = halves[hh]
        in_maps.append({"xt": xts[b], "wqt": wqt, "wkt": wkt, "wvt": wvt,
                        "wot": wot})
    res = run_bass_kernel_spmd(nc, in_maps, core_ids=list(range(8)))
    bo2 = np.asarray(bo, np.float32).reshape(1, D)
    out = np.empty((B, T, D), dtype=np.float32)
    for b in range(B):
        out[b] = (np.asarray(res.results[2 * b]["y"], np.float32)
                  + np.asarray(res.results[2 * b + 1]["y"], np.float32) + bo2)
    return out
